# revision 38
# baseline (speedup 1.0000x reference)
"""Trainium2 Bass kernel for nn_CINTransform: out[b,h,f] = sum_ij w[h,i,j]*in1[b,i,f]*in2[b,j,f].

Sharding: data-parallel over batch B=2048 across 8 NeuronCores (256 batches
per core); the small weight is replicated.

Per-core algorithm ("selector matmul" form), processing batches in quads
(4 per step, moving dim N = 4*F = 512):
  phase A (PE):   L_c[(h2,j), (b4,f)] = sum_i Wc[i,(h2,j)] * in1[b,i,f]
                  for 8 chunks c (each 2 h's), grouped 2 chunks per PSUM tile
  copy  (ACT):    groups 0-2: l3 = bf16(L)  (PSUM -> SBUF, 3 ops of FD=1024);
                  group 3 skips the copy (PSUM-direct 1x multiply on DVE)
  mult  (DVE):    tmp = l3 * in2dup (2x bf16, one op per copied group)
  phase B (PE):   psum_out[(par,h),(b4,f)] += sel_{c,par}^T @ tmp_c  (8
                  accumulated MMs per quad; the 0/1 selector contracts j on
                  the PE; two consecutive quads share one [32,N] accumulator
                  via zero-padded selectors)
  copy  (DVE):    out_sb pair-slice = psum_out (once per quad pair)

Phase-A MMs have K=64 (the i-contraction), so chunk-pairs run CONCURRENTLY
as two row-tiles of the PE array (tile_position (0,0)/(64,0), in1 duplicated
into both partition halves) - this nearly halves phase-A PE time.

Measured on 8x trn2 NeuronCores: ~259 us HW exec, rel err ~4.3e-3 (bf16).
Engine budget per quad (512 outputs): DVE ~3.7us (the wall, 91% busy),
ACT ~3.3us, PE ~3.0us after row-tiling.
Output leaves the device as [H, BLOC, F]; the host transposes back to [b,h,f].
"""

import numpy as np
import ml_dtypes

import concourse.bacc as bacc
import concourse.mybir as mybir
import concourse.tile as tile
from concourse import bass_utils


def _install_ntff_shim():
    """The image's antenv stub lacks axon_hooks; inject one so trace=True works."""
    import sys
    import types

    if "antenv.axon_hooks" in sys.modules:
        return
    try:
        from trn_agent_boot.trn_boot import _ntff_profile_via_ctypes

        hook = _ntff_profile_via_ctypes("/opt/axon/libaxon_pjrt.so")
    except Exception:
        hook = None
    mod = types.ModuleType("antenv.axon_hooks")
    mod.get_axon_ntff_profile_hook = lambda: hook
    mod.set_axon_ntff_profile_hook = lambda h: None
    sys.modules["antenv.axon_hooks"] = mod


_install_ntff_shim()

B, H, D1, D2, F = 2048, 16, 64, 64, 128
NCORES = 8
BLOC = B // NCORES  # 256
QB = 4  # batches per quad (moving dim N = QB*F = 512)
NQ = BLOC // QB  # 64 quads
NCHUNK = 8  # (h2, j) chunks of the (h, j) = 1024 axis
NACT = 6  # chunks copied via ACT (rest go PSUM-direct on DVE)
OBQ = 8  # quads per output DMA (32 batches)

_nc_cache = {}


def _build_nc():
    if "nc" in _nc_cache:
        return _nc_cache["nc"]
    nc = bacc.Bacc("TRN2", target_bir_lowering=False)
    bf16 = mybir.dt.bfloat16
    f32 = mybir.dt.float32
    N = QB * F  # 512

    in1_d = nc.dram_tensor("in1q", [NQ, D1, N], bf16, kind="ExternalInput")
    in2_d = nc.dram_tensor("in2q", [NQ, D2, N], bf16, kind="ExternalInput")
    # chunk-pairs stacked in partitions: rows 0-63 = chunk 2g, 64-127 = chunk 2g+1
    w_d = nc.dram_tensor("w", [2 * D1, (H * D2) // 2], bf16, kind="ExternalInput")
    # per (chunk, quad-parity): [128, 32] selector, zero rows for the other parity
    sel_d = nc.dram_tensor(
        "sel", [2 * D2, NCHUNK * 2 * 2 * H], bf16, kind="ExternalInput"
    )
    out_d = nc.dram_tensor("out", [H, BLOC, F], f32, kind="ExternalOutput")

    with tile.TileContext(nc) as tc:
        with (
            tc.tile_pool(name="const", bufs=1) as constp,
            tc.tile_pool(name="io", bufs=6) as io,
            tc.tile_pool(name="mid", bufs=4) as mid,
            tc.tile_pool(name="outp", bufs=2) as outp,
            tc.tile_pool(name="psl", bufs=2, space="PSUM") as psl,
            tc.tile_pool(name="pso", bufs=4, space="PSUM") as pso,
        ):
            w_sb = constp.tile([2 * D1, (H * D2) // 2], bf16)
            nc.scalar.dma_start(out=w_sb[:], in_=w_d[:])
            sel_sb = constp.tile([2 * D2, NCHUNK * 2 * 2 * H], bf16)
            nc.sync.dma_start(out=sel_sb[:], in_=sel_d[:])

            for qo in range(NQ // OBQ):
                # rows (parity, h) for quad-pairs; free (pair, b4*f)
                out_sb = outp.tile([2 * H, OBQ // 2, N], f32)
                for qi in range(OBQ):
                    q = qo * OBQ + qi
                    in1_sb = io.tile([2 * D1, N], bf16, tag="in1")
                    nc.sync.dma_start(out=in1_sb[0:D1, :], in_=in1_d[q])
                    nc.sync.dma_start(out=in1_sb[D1 : 2 * D1, :], in_=in1_d[q])
                    in2_sb = io.tile([2 * D2, N], bf16, tag="in2")
                    nc.sync.dma_start(out=in2_sb[0:D2, :], in_=in2_d[q])
                    nc.sync.dma_start(out=in2_sb[D2 : 2 * D2, :], in_=in2_d[q])

                    # phase A: 4 groups of 2 chunks; the PSUM-direct
                    # group (g3) first so its multiply + MM2s unlock early
                    in2_bc = in2_sb[:].unsqueeze(1)
                    gorder = [3, 0, 1, 2]
                    groups = {}
                    l3_sb = mid.tile([2 * D2, NACT, N], bf16, tag="l3")
                    tmp123 = mid.tile([2 * D2, NACT, N], bf16, tag="tmp123")
                    tmp4 = mid.tile([2 * D2, 2, N], bf16, tag="tmp4")
                    for g in gorder:
                        psum_l = psl.tile([2 * D2, 2, N], f32)
                        for k in range(2):
                            nc.tensor.matmul(
                                psum_l[:, k, :],
                                w_sb[k * D1 : (k + 1) * D1, g * 128 : (g + 1) * 128],
                                in1_sb[k * D1 : (k + 1) * D1, :],
                                tile_position=(k * D1, 0),
                            )
                        groups[g] = psum_l
                        if g == 3:
                            nc.vector.tensor_mul(
                                out=tmp4[:],
                                in0=psum_l[:],
                                in1=in2_bc.broadcast_to([2 * D2, 2, N]),
                            )
                        else:
                            nc.scalar.copy(
                                l3_sb[:, 2 * g : 2 * g + 2, :], psum_l[:]
                            )
                            nc.vector.tensor_mul(
                                out=tmp123[:, 2 * g : 2 * g + 2, :],
                                in0=l3_sb[:, 2 * g : 2 * g + 2, :],
                                in1=in2_bc.broadcast_to([2 * D2, 2, N]),
                            )

                    # two consecutive quads accumulate into one [32, N] psum
                    # (rows (parity, h)); one copy out per pair
                    if qi % 2 == 0:
                        psum_out = pso.tile([2 * H, N], f32)
                        _pair_pso = psum_out
                    else:
                        psum_out = _pair_pso
                    par = qi % 2
                    corder = [6, 7, 0, 1, 2, 3, 4, 5]
                    for ci, c in enumerate(corder):
                        rhs = (
                            tmp123[:, c, :]
                            if c < NACT
                            else tmp4[:, c - NACT, :]
                        )
                        s0 = (c * 2 + par) * 2 * H
                        nc.tensor.matmul(
                            psum_out[:],
                            sel_sb[:, s0 : s0 + 2 * H],
                            rhs,
                            start=(par == 0 and ci == 0),
                            stop=(par == 1 and ci == NCHUNK - 1),
                        )
                    if qi % 2 == 1:
                        nc.vector.tensor_copy(
                            out=out_sb[:, qi // 2, :], in_=psum_out[:]
                        )
                blk = out_d[:, qo * OBQ * QB : (qo + 1) * OBQ * QB, :].rearrange(
                    "h (qp e bb) f -> h qp e bb f", qp=OBQ // 2, e=2, bb=QB
                )
                for e in range(2):
                    nc.sync.dma_start(
                        out=blk[:, :, e, :, :],
                        in_=out_sb[e * H : (e + 1) * H, :, :].rearrange(
                            "h q (bb f) -> h q bb f", bb=QB
                        ),
                    )
    nc.compile()
    _nc_cache["nc"] = nc
    return nc


def _prep_inputs(input1, input2, weight):
    """Host-side layout prep: cast to bf16, shard over B, pack quads."""
    bf = ml_dtypes.bfloat16
    N = QB * F

    def packq(x):
        x = x.astype(bf).reshape(NCORES, NQ, QB, x.shape[1], F)
        x = x.transpose(0, 1, 3, 2, 4).reshape(NCORES, NQ, x.shape[3], N)
        return np.ascontiguousarray(x)

    in1q = packq(input1)
    in2q = packq(input2)
    warr = weight.astype(bf).transpose(1, 0, 2).reshape(D1, H * D2)
    # stack chunk-pairs in partitions: [2*D1, 512]; w2[0:64, g-slice] = chunk 2g
    w2 = np.zeros((2 * D1, (H * D2) // 2), dtype=bf)
    for g in range(NCHUNK // 2):
        w2[0:D1, g * 128 : (g + 1) * 128] = warr[:, (2 * g) * 128 : (2 * g + 1) * 128]
        w2[D1 : 2 * D1, g * 128 : (g + 1) * 128] = warr[
            :, (2 * g + 1) * 128 : (2 * g + 2) * 128
        ]
    warr = np.ascontiguousarray(w2)
    # sel[(e,j), (c, par, m)]: m = par*H + h with h = 2c+e; zero rows otherwise
    sel = np.zeros((2 * D2, NCHUNK, 2, 2 * H), dtype=bf)
    for c in range(NCHUNK):
        for par in range(2):
            for e in range(2):
                sel[e * D2 : (e + 1) * D2, c, par, par * H + 2 * c + e] = 1.0
    sel = np.ascontiguousarray(sel.reshape(2 * D2, NCHUNK * 2 * 2 * H))
    in_maps = []
    for cix in range(NCORES):
        in_maps.append(
            {"in1q": in1q[cix], "in2q": in2q[cix], "w": warr, "sel": sel}
        )
    return in_maps


def kernel(input1, input2, weight, _trace=False):
    nc = _build_nc()
    in_maps = _prep_inputs(input1, input2, weight)
    res = None
    last_exc = None
    for _attempt in range(3):
        try:
            res = bass_utils.run_bass_kernel_spmd(
                nc, in_maps, core_ids=list(range(NCORES)), trace=_trace
            )
            break
        except Exception as e:  # transient NRT device errors: retry
            last_exc = e
            import time as _time

            _time.sleep(2.0)
    if res is None:
        raise last_exc
    outs = []
    for c in range(NCORES):
        o = res.results[c]["out"]  # [H, BLOC, F]
        outs.append(np.ascontiguousarray(o.transpose(1, 0, 2)))  # [BLOC, H, F]
    full = np.concatenate(outs, axis=0).astype(np.float32)  # [B, H, F]
    if _trace:
        kernel.last_results = res
    return full


# revision 39
# speedup vs baseline: 1.0878x; 1.0878x over previous
"""Trainium2 Bass kernel for nn_CINTransform: out[b,h,f] = sum_ij w[h,i,j]*in1[b,i,f]*in2[b,j,f].

Sharding: data-parallel over batch B=2048 across 8 NeuronCores (256 batches
per core); the small weight is replicated.

Per-core algorithm ("selector matmul" form), processing batches in quads
(4 per step, moving dim N = 4*F = 512):
  phase A (PE):   L_c[(h2,j), (b4,f)] = sum_i Wc[i,(h2,j)] * in1[b,i,f]
                  for 8 chunks c (each 2 h's), grouped 2 chunks per PSUM tile
  copy  (ACT):    groups 0-2: l3 = bf16(L)  (PSUM -> SBUF, 3 ops of FD=1024);
                  group 3 skips the copy (PSUM-direct 1x multiply on DVE)
  mult  (DVE):    tmp = l3 * in2dup (2x bf16, one op per copied group)
  phase B (PE):   psum_out[(par,h),(b4,f)] += sel_{c,par}^T @ tmp_c  (8
                  accumulated MMs per quad; the 0/1 selector contracts j on
                  the PE; two consecutive quads share one [32,N] accumulator
                  via zero-padded selectors)
  copy  (DVE):    out_sb pair-slice = psum_out (once per quad pair)

Phase-A MMs have K=64 (the i-contraction), so chunk-pairs run CONCURRENTLY
as two row-tiles of the PE array (tile_position (0,0)/(64,0), in1 duplicated
into both partition halves) - this nearly halves phase-A PE time.

Measured on 8x trn2 NeuronCores: ~259 us HW exec, rel err ~4.3e-3 (bf16).
Engine budget per quad (512 outputs): DVE ~3.7us (the wall, 91% busy),
ACT ~3.3us, PE ~3.0us after row-tiling.
Output leaves the device as [H, BLOC, F]; the host transposes back to [b,h,f].
"""

import numpy as np
import ml_dtypes

import concourse.bacc as bacc
import concourse.mybir as mybir
import concourse.tile as tile
from concourse import bass_utils


def _install_ntff_shim():
    """The image's antenv stub lacks axon_hooks; inject one so trace=True works."""
    import sys
    import types

    if "antenv.axon_hooks" in sys.modules:
        return
    try:
        from trn_agent_boot.trn_boot import _ntff_profile_via_ctypes

        hook = _ntff_profile_via_ctypes("/opt/axon/libaxon_pjrt.so")
    except Exception:
        hook = None
    mod = types.ModuleType("antenv.axon_hooks")
    mod.get_axon_ntff_profile_hook = lambda: hook
    mod.set_axon_ntff_profile_hook = lambda h: None
    sys.modules["antenv.axon_hooks"] = mod


_install_ntff_shim()

B, H, D1, D2, F = 2048, 16, 64, 64, 128
NCORES = 8
BLOC = B // NCORES  # 256
QB = 4  # batches per quad (moving dim N = QB*F = 512)
NQ = BLOC // QB  # 64 quads
NCHUNK = 8  # (h2, j) chunks of the (h, j) = 1024 axis
NACT = 6  # chunks copied via ACT (rest go PSUM-direct on DVE)
OBQ = 8  # quads per output DMA (32 batches)

_nc_cache = {}


def _build_nc():
    if "nc" in _nc_cache:
        return _nc_cache["nc"]
    nc = bacc.Bacc("TRN2", target_bir_lowering=False)
    bf16 = mybir.dt.bfloat16
    f32 = mybir.dt.float32
    N = QB * F  # 512

    in1_d = nc.dram_tensor("in1q", [NQ, D1, N], bf16, kind="ExternalInput")
    in2_d = nc.dram_tensor("in2q", [NQ, D2, N], bf16, kind="ExternalInput")
    # chunk-pairs stacked in partitions: rows 0-63 = chunk 2g, 64-127 = chunk 2g+1
    w_d = nc.dram_tensor("w", [2 * D1, (H * D2) // 2], bf16, kind="ExternalInput")
    # per (chunk, quad-parity): [128, 32] selector, zero rows for the other parity
    sel_d = nc.dram_tensor(
        "sel", [2 * D2, NCHUNK * 2 * 2 * H], bf16, kind="ExternalInput"
    )
    out_d = nc.dram_tensor("out", [H, BLOC, F], f32, kind="ExternalOutput")

    with tile.TileContext(nc) as tc:
        with (
            tc.tile_pool(name="const", bufs=1) as constp,
            tc.tile_pool(name="io", bufs=6) as io,
            tc.tile_pool(name="mid", bufs=4) as mid,
            tc.tile_pool(name="outp", bufs=2) as outp,
            tc.tile_pool(name="psl", bufs=3, space="PSUM") as psl,
            tc.tile_pool(name="pso", bufs=2, space="PSUM") as pso,
        ):
            w_sb = constp.tile([2 * D1, (H * D2) // 2], bf16)
            nc.scalar.dma_start(out=w_sb[:], in_=w_d[:])
            sel_sb = constp.tile([2 * D2, NCHUNK * 2 * 2 * H], bf16)
            nc.scalar.dma_start(out=sel_sb[:], in_=sel_d[:])

            for qo in range(NQ // OBQ):
                # rows (parity, h) for quad-pairs; free (pair, b4*f)
                out_sb = outp.tile([2 * H, OBQ // 2, N], f32)
                for qi in range(OBQ):
                    q = qo * OBQ + qi
                    in1_sb = io.tile([2 * D1, N], bf16, tag="in1")
                    nc.sync.dma_start(out=in1_sb[0:D1, :], in_=in1_d[q])
                    nc.sync.dma_start(out=in1_sb[D1 : 2 * D1, :], in_=in1_d[q])
                    in2_sb = io.tile([2 * D2, N], bf16, tag="in2")
                    nc.sync.dma_start(out=in2_sb[0:D2, :], in_=in2_d[q])
                    nc.sync.dma_start(out=in2_sb[D2 : 2 * D2, :], in_=in2_d[q])

                    # phase A: 4 groups of 2 chunks; the PSUM-direct
                    # group (g3) first so its multiply + MM2s unlock early
                    in2_bc = in2_sb[:].unsqueeze(1)
                    gorder = [3, 0, 1, 2]
                    groups = {}
                    l3_sb = mid.tile([2 * D2, NACT, N], bf16, tag="l3")
                    tmp123 = mid.tile([2 * D2, NACT, N], bf16, tag="tmp123")
                    tmp4 = mid.tile([2 * D2, 2, N], bf16, tag="tmp4")
                    for g in gorder:
                        psum_l = psl.tile([2 * D2, 2, N], f32)
                        for k in range(2):
                            nc.tensor.matmul(
                                psum_l[:, k, :],
                                w_sb[k * D1 : (k + 1) * D1, g * 128 : (g + 1) * 128],
                                in1_sb[k * D1 : (k + 1) * D1, :],
                                tile_position=(k * D1, 0),
                            )
                        groups[g] = psum_l
                        if g == 3:
                            nc.vector.tensor_mul(
                                out=tmp4[:],
                                in0=psum_l[:],
                                in1=in2_bc.broadcast_to([2 * D2, 2, N]),
                            )
                        else:
                            nc.scalar.copy(
                                l3_sb[:, 2 * g : 2 * g + 2, :], psum_l[:]
                            )
                            nc.vector.tensor_mul(
                                out=tmp123[:, 2 * g : 2 * g + 2, :],
                                in0=l3_sb[:, 2 * g : 2 * g + 2, :],
                                in1=in2_bc.broadcast_to([2 * D2, 2, N]),
                            )

                    # two consecutive quads accumulate into one [32, N] psum
                    # (rows (parity, h)); one copy out per pair
                    if qi % 2 == 0:
                        psum_out = pso.tile([2 * H, N], f32)
                        _pair_pso = psum_out
                    else:
                        psum_out = _pair_pso
                    par = qi % 2
                    corder = [6, 7, 0, 1, 2, 3, 4, 5]
                    for ci, c in enumerate(corder):
                        rhs = (
                            tmp123[:, c, :]
                            if c < NACT
                            else tmp4[:, c - NACT, :]
                        )
                        s0 = (c * 2 + par) * 2 * H
                        nc.tensor.matmul(
                            psum_out[:],
                            sel_sb[:, s0 : s0 + 2 * H],
                            rhs,
                            start=(par == 0 and ci == 0),
                            stop=(par == 1 and ci == NCHUNK - 1),
                        )
                    if qi % 2 == 1:
                        nc.vector.tensor_copy(
                            out=out_sb[:, qi // 2, :], in_=psum_out[:]
                        )
                blk = out_d[:, qo * OBQ * QB : (qo + 1) * OBQ * QB, :].rearrange(
                    "h (qp e bb) f -> h qp e bb f", qp=OBQ // 2, e=2, bb=QB
                )
                for e in range(2):
                    nc.sync.dma_start(
                        out=blk[:, :, e, :, :],
                        in_=out_sb[e * H : (e + 1) * H, :, :].rearrange(
                            "h q (bb f) -> h q bb f", bb=QB
                        ),
                    )
    nc.compile()
    _nc_cache["nc"] = nc
    return nc


def _prep_inputs(input1, input2, weight):
    """Host-side layout prep: cast to bf16, shard over B, pack quads."""
    bf = ml_dtypes.bfloat16
    N = QB * F

    def packq(x):
        x = x.astype(bf).reshape(NCORES, NQ, QB, x.shape[1], F)
        x = x.transpose(0, 1, 3, 2, 4).reshape(NCORES, NQ, x.shape[3], N)
        return np.ascontiguousarray(x)

    in1q = packq(input1)
    in2q = packq(input2)
    warr = weight.astype(bf).transpose(1, 0, 2).reshape(D1, H * D2)
    # stack chunk-pairs in partitions: [2*D1, 512]; w2[0:64, g-slice] = chunk 2g
    w2 = np.zeros((2 * D1, (H * D2) // 2), dtype=bf)
    for g in range(NCHUNK // 2):
        w2[0:D1, g * 128 : (g + 1) * 128] = warr[:, (2 * g) * 128 : (2 * g + 1) * 128]
        w2[D1 : 2 * D1, g * 128 : (g + 1) * 128] = warr[
            :, (2 * g + 1) * 128 : (2 * g + 2) * 128
        ]
    warr = np.ascontiguousarray(w2)
    # sel[(e,j), (c, par, m)]: m = par*H + h with h = 2c+e; zero rows otherwise
    sel = np.zeros((2 * D2, NCHUNK, 2, 2 * H), dtype=bf)
    for c in range(NCHUNK):
        for par in range(2):
            for e in range(2):
                sel[e * D2 : (e + 1) * D2, c, par, par * H + 2 * c + e] = 1.0
    sel = np.ascontiguousarray(sel.reshape(2 * D2, NCHUNK * 2 * 2 * H))
    in_maps = []
    for cix in range(NCORES):
        in_maps.append(
            {"in1q": in1q[cix], "in2q": in2q[cix], "w": warr, "sel": sel}
        )
    return in_maps


def kernel(input1, input2, weight, _trace=False):
    nc = _build_nc()
    in_maps = _prep_inputs(input1, input2, weight)
    res = None
    last_exc = None
    for _attempt in range(3):
        try:
            res = bass_utils.run_bass_kernel_spmd(
                nc, in_maps, core_ids=list(range(NCORES)), trace=_trace
            )
            break
        except Exception as e:  # transient NRT device errors: retry
            last_exc = e
            import time as _time

            _time.sleep(2.0)
    if res is None:
        raise last_exc
    outs = []
    for c in range(NCORES):
        o = res.results[c]["out"]  # [H, BLOC, F]
        outs.append(np.ascontiguousarray(o.transpose(1, 0, 2)))  # [BLOC, H, F]
    full = np.concatenate(outs, axis=0).astype(np.float32)  # [B, H, F]
    if _trace:
        kernel.last_results = res
    return full
